# revision 1
# baseline (speedup 1.0000x reference)
"""Trainium2 Bass kernel for MixedCausalAttention (16 heads, d=1024, L_S=4096, L_NS=64).

Sharding: tensor-parallel over heads - 2 heads per core x 8 cores.
Each core computes qkv projections (shared W_S for S tokens, per-token W_NS for
NS tokens) for its 2 heads, causal attention, and a partial W_out product over
its 128 output feature rows. The host sums the 8 partial (2112, 1024) outputs.

On-device layout strategy: activations flow transposed ([feature, seq]) so
Q^T/K^T/V^T come straight out of the PE in matmul-ready form. Attention scores
are computed transposed (scores^T[k, q]); softmax needs no partition-dim
reduction because the denominator is produced by an extra all-ones column
appended to the V stationary operand of the attn@V matmul. Causal masking is an
affine_select staircase applied only to the diagonal-crossing tiles; k-chunks
fully above the diagonal are skipped entirely.

Dtypes: fp32r (TF32-like) everywhere on the main path (full PE speed at free
dim >= 256, ~1e-3 precision); bf16 only for the 805MB W_NS stream and its tiny
x_NS stationary, halving the dominant DMA cost.
"""

import os
import sys
import math
from concurrent.futures import ThreadPoolExecutor

for _p in ("/opt/trn_rl_repo", "/root/.axon_site/_ro/trn_rl_repo"):
    if os.path.isdir(_p) and _p not in sys.path:
        sys.path.insert(0, _p)

import numpy as np
import ml_dtypes

import concourse.bass as bass
import concourse.mybir as mybir
import concourse.tile as tile
from concourse import bacc
from concourse.bass_utils import run_bass_kernel_spmd
from concourse.masks import make_identity

F32 = mybir.dt.float32
F32R = mybir.dt.float32r
BF16 = mybir.dt.bfloat16

N_CORES = 8
D = 1024
H = 16
DH = 64
HPC = H // N_CORES          # heads per core = 2
O3 = 3 * DH * HPC           # 384 qkv output cols per core
LNS = 64
LS = 4096
QS = 2048                   # query_start
LQ = LS - QS + LNS          # 2112 queries
NCH = D // 128              # 8 contraction chunks
ST = 512                    # s-tile width for projections
QT = 512                    # q-tile width for attention
SCALE = DH ** -0.5


def build_program(repeat=1):
    nc = bacc.Bacc("TRN2", target_bir_lowering=False, debug=False,
                   num_devices=N_CORES)

    xt_d = nc.dram_tensor("xt", [128, NCH, LS], F32R, kind="ExternalInput")
    xnst_d = nc.dram_tensor("xnst", [128, NCH, LNS], BF16, kind="ExternalInput")
    ws_d = nc.dram_tensor("ws", [128, NCH, O3], F32R, kind="ExternalInput")
    wns_d = nc.dram_tensor("wns", [LNS, 128, NCH, O3], BF16, kind="ExternalInput")
    wout0_d = nc.dram_tensor("wout0", [64, D], F32R, kind="ExternalInput")
    wout1_d = nc.dram_tensor("wout1", [64, D], F32R, kind="ExternalInput")
    vones_d = nc.dram_tensor("vones", [128, 64], F32R, kind="ExternalInput")
    o_d = nc.dram_tensor("o", [LQ, D], F32, kind="ExternalOutput")

    n_kc_s = LS // 128       # 32 S key chunks
    n_kc = n_kc_s + 1        # + NS chunk
    lqs = LS - QS            # 2048 S-query columns

    with tile.TileContext(nc) as tc:
      for _rep in range(repeat):
        import contextlib
        ctx = contextlib.ExitStack()
        with ctx:
            const = ctx.enter_context(tc.tile_pool(name="const", bufs=1))
            store = ctx.enter_context(tc.tile_pool(name="store", bufs=1))

            # --- constants ---
            ws_sb = const.tile([128, NCH, O3], F32R)
            for ci in range(NCH):
                nc.sync.dma_start(out=ws_sb[:, ci, :], in_=ws_d.ap()[:, ci, :])
            xpool = ctx.enter_context(tc.tile_pool(name="xpool", bufs=2))
            xt0_t = xpool.tile([128, NCH, ST], F32R, tag="xt", name="xt0_t")
            for ci in range(NCH):
                nc.sync.dma_start(out=xt0_t[:, ci, :], in_=xt_d.ap()[:, ci, 0:ST])
            xnst_sb = const.tile([128, NCH, LNS], BF16)
            nc.sync.dma_start(out=xnst_sb, in_=xnst_d.ap())
            wout0_sb = const.tile([64, D], F32R)
            nc.sync.dma_start(out=wout0_sb, in_=wout0_d.ap())
            wout1_sb = const.tile([64, D], F32R)
            nc.sync.dma_start(out=wout1_sb, in_=wout1_d.ap())
            ident_sb = const.tile([64, 64], F32)
            make_identity(nc, ident_sb[:, :])
            ones64_sb = const.tile([65, 64], F32)
            nc.gpsimd.dma_start(out=ones64_sb[:, :], in_=vones_d.ap()[0:65, 0:64])

            # --- persistent activation storage ---
            qt_s = store.tile([128, lqs], F32R)     # Q^T, S part (h0 rows 0-63, h1 64-127)
            qt_ns = store.tile([128, LNS], F32R)    # Q^T, NS part
            kt_s = store.tile([128, LS], F32R)      # K^T, S part
            kt_ns = store.tile([128, LNS], F32R)    # K^T, NS part
            vt_sb = store.tile([128, LS], F32)      # V^T (pre-transpose staging)
            vt1_lo = store.tile([64, LS], F32)      # V^T head1 moved to partitions 0-63
            v_s = [store.tile([128, n_kc_s, 65], F32R, name=f"v_s{h}") for h in range(2)]
            v_ns = [store.tile([64, 65], F32R, name=f"v_ns{h}") for h in range(2)]
            qkvns_sb = store.tile([LNS, O3], F32)   # natural-layout NS qkv rows

            # ones columns for the denominator trick (memset can't emit f32r,
            # so DMA them in from a tiny all-ones DRAM constant)
            for h in range(2):
                nc.sync.dma_start(out=v_s[h][:, :, 64:65],
                                  in_=vones_d.ap()[:, 0:n_kc_s])
                nc.sync.dma_start(out=v_ns[h][:, 64:65],
                                  in_=vones_d.ap()[0:64, 0:1])

            # ---------------- NS-token projections (emitted interleaved) ----
            wnspool = ctx.enter_context(tc.tile_pool(name="wnspool", bufs=4))
            psNS = ctx.enter_context(tc.tile_pool(name="psNS", bufs=2, space="PSUM"))

            nsstage = ctx.enter_context(tc.tile_pool(name="nsstage", bufs=3))

            def ns_emitter():
                for n in range(LNS):
                    wns_t = wnspool.tile([128, NCH, O3], BF16, tag="wns")
                    nc.sync.dma_start(out=wns_t, in_=wns_d.ap()[n])
                    psn = psNS.tile([1, O3], F32, tag="psNS")
                    for ci in range(NCH):
                        nc.tensor.matmul(
                            psn[:, :], lhsT=xnst_sb[:, ci, n:n + 1],
                            rhs=wns_t[:, ci, :],
                            start=(ci == 0), stop=(ci == NCH - 1))
                    stg = nsstage.tile([1, O3], F32, tag="nsstg")
                    nc.vector.tensor_copy(out=stg[:, :], in_=psn[:, :])
                    # engines can't write arbitrary partitions; a tiny DMA can
                    nc.gpsimd.dma_start(out=qkvns_sb[n:n + 1, :], in_=stg[:, :])
                    yield
                # finalize: Q_NS^T / K_NS^T via PE transpose, V_NS natural copies
                for part, dest in ((0, qt_ns), (1, kt_ns)):
                    pst2 = psNS.tile([128, 64], F32, tag="psNS")
                    nc.tensor.transpose(
                        pst2[:, :], qkvns_sb[0:64, part * 128:(part + 1) * 128],
                        ident_sb[:, :])
                    nc.vector.tensor_copy(out=dest[:, :], in_=pst2[:, :])
                for h in range(2):
                    nc.vector.tensor_copy(
                        out=v_ns[h][0:64, 0:64],
                        in_=qkvns_sb[0:64, 256 + h * 64:256 + (h + 1) * 64])
                while True:
                    yield

            ns_gen = ns_emitter()
            ns_left = LNS + 1  # tokens + finalize step

            # ---------------- stage A: S-token projections ----------------
            with tc.tile_pool(name="psA", bufs=2, space="PSUM") as psA:
                for st in range(LS // ST):
                    s0 = st * ST
                    if st == 0:
                        xt_t = xt0_t
                    else:
                        xt_t = xpool.tile([128, NCH, ST], F32R, tag="xt", name="xt_t")
                        nc.sync.dma_start(out=xt_t, in_=xt_d.ap()[:, :, s0:s0 + ST])
                    jobs = [(1, kt_s, s0), (2, vt_sb, s0)]
                    if s0 >= QS:
                        jobs.append((0, qt_s, s0 - QS))
                    for mi, dest, dcol in jobs:
                        ps = psA.tile([128, ST], F32, tag="psA")
                        for ci in range(NCH):
                            nc.tensor.matmul(
                                ps[:, :],
                                lhsT=ws_sb[:, ci, mi * 128:(mi + 1) * 128],
                                rhs=xt_t[:, ci, :],
                                start=(ci == 0), stop=(ci == NCH - 1))
                        nc.vector.tensor_copy(out=dest[:, dcol:dcol + ST], in_=ps[:, :])
                    if ns_left > 0:
                        next(ns_gen)
                        ns_left -= 1

                # V^T -> V (natural) via PE transposes; move h1 rows to base 0 first
                nc.sync.dma_start(out=vt1_lo[:, :], in_=vt_sb[64:128, :])
                for h in range(2):
                    src_t = vt_sb if h == 0 else vt1_lo
                    for kc in range(n_kc_s):
                        pst = psA.tile([128, 64], F32, tag="psT")
                        nc.tensor.transpose(
                            pst[:, :], src_t[0:64, kc * 128:(kc + 1) * 128],
                            ident_sb[:, :])
                        nc.vector.tensor_copy(out=v_s[h][:, kc, 0:64], in_=pst[:, :])
                        if kc % 4 == 0 and ns_left > 0:
                            next(ns_gen)
                            ns_left -= 1

            # ---------------- main attention loop ----------------
            expool = ctx.enter_context(tc.tile_pool(name="expool", bufs=6))
            recpool = ctx.enter_context(tc.tile_pool(name="recpool", bufs=2))
            bcpool = ctx.enter_context(tc.tile_pool(name="bcpool", bufs=2))
            avtnpool = ctx.enter_context(tc.tile_pool(name="avtnpool", bufs=2))
            outpool = ctx.enter_context(tc.tile_pool(name="outpool", bufs=2))
            psS = ctx.enter_context(tc.tile_pool(name="psS", bufs=3, space="PSUM"))
            psAV = ctx.enter_context(tc.tile_pool(name="psAV", bufs=2, space="PSUM"))
            psMisc = ctx.enter_context(tc.tile_pool(name="psMisc", bufs=1, space="PSUM"))

            ns_acc = 0.0
            q_tiles = [(q0, min(QT, LQ - q0)) for q0 in range(0, LQ, QT)]
            for qt_i, (q0, qw) in enumerate(q_tiles):
                kc_count = min((QS + q0 + qw - 1) // 128 + 1, n_kc)
                is_last_qt = (q0 >= lqs)
                if is_last_qt:
                    # drain any remaining NS emission before NS data is needed
                    while ns_left > 0:
                        next(ns_gen)
                        ns_left -= 1
                ps_av = [psAV.tile([65, QT], F32, tag="psAV", name=f"ps_av{h}")
                         for h in range(2)]
                for kc in range(kc_count):
                    is_ns_chunk = (kc == n_kc_s)
                    kw = LNS if is_ns_chunk else 128
                    for h in range(2):
                        hs = slice(h * 64, h * 64 + 64)
                        if is_ns_chunk:
                            k_src = kt_ns[hs, 0:kw]
                        else:
                            k_src = kt_s[hs, kc * 128:kc * 128 + kw]
                        if is_last_qt:
                            q_src = qt_ns[hs, q0 - lqs:q0 - lqs + qw]
                        else:
                            q_src = qt_s[hs, q0:q0 + qw]
                        ps_s = psS.tile([128, QT], F32, tag="psS")
                        nc.tensor.matmul(ps_s[0:kw, 0:qw], lhsT=k_src, rhs=q_src,
                                         start=True, stop=True)
                        ex = expool.tile([128, QT], F32R, tag="exp")
                        nc.scalar.activation(
                            out=ex[0:kw, 0:qw], in_=ps_s[0:kw, 0:qw],
                            func=mybir.ActivationFunctionType.Exp, scale=SCALE)
                        if 128 * kc + kw - 1 > QS + q0:
                            # causal staircase: keep iff q_pos - k_pos >= 0
                            nc.gpsimd.affine_select(
                                out=ex[0:kw, 0:qw], in_=ex[0:kw, 0:qw],
                                compare_op=mybir.AluOpType.is_ge, fill=0.0,
                                base=QS + q0 - 128 * kc, channel_multiplier=-1,
                                pattern=[[1, qw]])
                        v_src = v_ns[h][0:kw, 0:65] if is_ns_chunk \
                            else v_s[h][0:kw, kc, 0:65]
                        nc.tensor.matmul(ps_av[h][0:65, 0:qw], lhsT=v_src,
                                         rhs=ex[0:kw, 0:qw],
                                         start=(kc == 0), stop=(kc == kc_count - 1))
                    if ns_left > 0:
                        next(ns_gen)
                        ns_left -= 1

                # normalize: divide attn output rows by the ones-column sums.
                # The [1, qw] reciprocal row (partition 64) is broadcast across
                # partitions 0-63 with a K=1 PE matmul against an all-ones
                # stationary column, then multiplied in on the DVE.
                avtn = []
                for h in range(2):
                    # stage the accumulator out of PSUM immediately so the AV
                    # bank frees for the next q-tile's accumulation
                    avu = recpool.tile([65, QT], F32, tag=f"avu{h}", name=f"avu{h}")
                    nc.vector.tensor_copy(out=avu[0:65, 0:qw], in_=ps_av[h][0:65, 0:qw])
                    rc = recpool.tile([65, QT], F32, tag="recip")
                    nc.vector.reciprocal(out=rc[64:65, 0:qw], in_=avu[64:65, 0:qw])
                    pbc = psMisc.tile([128, 512], F32, tag="psMisc", name="pbc")
                    nc.tensor.matmul(pbc[0:64, 0:qw], lhsT=ones64_sb[64:65, 0:64],
                                     rhs=rc[64:65, 0:qw], start=True, stop=True)
                    bc = bcpool.tile([64, QT], F32, tag="bcast")
                    nc.vector.tensor_copy(out=bc[0:64, 0:qw], in_=pbc[0:64, 0:qw])
                    av = avtnpool.tile([64, QT], F32R, tag=f"avtn{h}", name=f"av{h}")
                    nc.vector.tensor_mul(av[0:64, 0:qw], avu[0:64, 0:qw],
                                         bc[0:64, 0:qw])
                    avtn.append(av)

                # partial W_out: out[q, :] = sum_h avtn_h.T @ wout_h
                for qs in range(math.ceil(qw / 128)):
                    qsw = min(128, qw - qs * 128)
                    ot = outpool.tile([128, D], F32, tag="out")
                    for e in range(2):
                        po = psMisc.tile([128, 512], F32, tag="psMisc", name="po")
                        nc.tensor.matmul(
                            po[0:qsw, :],
                            lhsT=avtn[0][0:64, qs * 128:qs * 128 + qsw],
                            rhs=wout0_sb[0:64, e * 512:(e + 1) * 512],
                            start=True, stop=False)
                        nc.tensor.matmul(
                            po[0:qsw, :],
                            lhsT=avtn[1][0:64, qs * 128:qs * 128 + qsw],
                            rhs=wout1_sb[0:64, e * 512:(e + 1) * 512],
                            start=False, stop=True)
                        nc.vector.tensor_copy(out=ot[0:qsw, e * 512:(e + 1) * 512],
                                              in_=po[0:qsw, :])
                    nc.sync.dma_start(
                        out=o_d.ap()[q0 + qs * 128:q0 + qs * 128 + qsw, :],
                        in_=ot[0:qsw, :])

    nc.compile()
    return nc


_NC_CACHE = {}


def _get_program():
    if "nc" not in _NC_CACHE:
        _NC_CACHE["nc"] = build_program()
    return _NC_CACHE["nc"]


def _prep_core(c, x2, W_S, W_NS, W_out):
    """Host-side shard prep for core c (heads 2c, 2c+1)."""
    h0 = 2 * c * DH
    cols = np.r_[h0:h0 + HPC * DH,
                 D + h0:D + h0 + HPC * DH,
                 2 * D + h0:2 * D + h0 + HPC * DH]
    ws = W_S[:, cols].reshape(NCH, 128, O3).transpose(1, 0, 2)
    ws = np.ascontiguousarray(ws, dtype=np.float32)
    wns = W_NS[:, :, cols].reshape(LNS, NCH, 128, O3).transpose(0, 2, 1, 3)
    wns = np.ascontiguousarray(wns.astype(ml_dtypes.bfloat16))
    wout0 = np.ascontiguousarray(W_out[h0:h0 + DH], dtype=np.float32)
    wout1 = np.ascontiguousarray(W_out[h0 + DH:h0 + 2 * DH], dtype=np.float32)
    return {"xt": x2, "xnst": _prep_core.xnst, "ws": ws, "wns": wns,
            "wout0": wout0, "wout1": wout1,
            "vones": np.ones((128, 64), dtype=np.float32)}


def kernel(x, W_S, W_NS, W_out, L_S=None, query_start=None, **_unused):
    x = np.asarray(x, dtype=np.float32)
    W_S = np.asarray(W_S, dtype=np.float32)
    W_NS = np.asarray(W_NS, dtype=np.float32)
    W_out = np.asarray(W_out, dtype=np.float32)
    if L_S is not None:
        assert int(L_S) == LS, f"kernel hardcodes L_S={LS}, got {int(L_S)}"
    if query_start is not None:
        assert int(query_start) == QS, \
            f"kernel hardcodes query_start={QS}, got {int(query_start)}"
    assert x.shape == (1, LS + LNS, D)

    nc = _get_program()

    xs = x[0]                                         # (4160, 1024)
    x2 = xs[:LS].T.reshape(NCH, 128, LS).transpose(1, 0, 2)
    x2 = np.ascontiguousarray(x2, dtype=np.float32)   # (128, 8, 4096)
    xnst = xs[LS:].T.reshape(NCH, 128, LNS).transpose(1, 0, 2)
    _prep_core.xnst = np.ascontiguousarray(xnst.astype(ml_dtypes.bfloat16))

    with ThreadPoolExecutor(max_workers=N_CORES) as ex:
        in_maps = list(ex.map(lambda c: _prep_core(c, x2, W_S, W_NS, W_out),
                              range(N_CORES)))

    res = None
    for attempt in range(3):
        try:
            res = run_bass_kernel_spmd(nc, in_maps, list(range(N_CORES)))
            break
        except Exception:
            if attempt == 2:
                raise
            # transient device wedges (NRT_EXEC_UNIT_UNRECOVERABLE) have been
            # observed to clear after the terminal resets the session
            import time
            time.sleep(100)
    out = np.zeros((LQ, D), dtype=np.float32)
    for r in res.results:
        out += r["o"]
    return out.reshape(1, LQ, D)


if __name__ == "__main__":
    rng = np.random.default_rng(0)
    ins = {
        "x": rng.standard_normal((1, LS + LNS, D), dtype=np.float32),
        "W_S": rng.standard_normal((D, 3 * D), dtype=np.float32) * 0.02,
        "W_NS": rng.standard_normal((LNS, D, 3 * D), dtype=np.float32) * 0.02,
        "W_out": rng.standard_normal((D, D), dtype=np.float32) * 0.03,
        "L_S": LS, "query_start": QS,
    }
    out = kernel(**ins)
    print("kernel out shape:", out.shape, "finite:", np.isfinite(out).all())



# revision 25
# speedup vs baseline: 1.6994x; 1.6994x over previous
"""Trainium2 Bass kernel for MixedCausalAttention (16 heads, d=1024, L_S=4096, L_NS=64).

Sharding: tensor-parallel over heads - 2 heads per core x 8 cores.
Each core computes qkv projections (shared W_S for S tokens, per-token W_NS for
NS tokens) for its 2 heads, causal attention, and a partial W_out product over
its 128 output feature rows. The host sums the 8 partial (2112, 1024) outputs.

Engine budget strategy (per core, timeline-sim calibrated):
- PE: S-token projections stream x^T (bf16) against W_S stationaries; V is
  produced in natural [s, dh] layout directly (x^T chunk as stationary) so no
  PE transposes are needed. The 805MB W_NS stream is fp8 (scaled 32x to dodge
  e4m3 subnormals) and consumed by DoubleRow matmuls (2 contraction chunks per
  pass at 0.5 cycles/row). A zero-padded selector stationary (xmask) places
  each NS token's projection in its own PSUM partition so 32 tokens accumulate
  into one bank, replacing 64 single-row staging copies with 2 wide ones.
- Act: exp over [128, 2heads, qw] two-bank PSUM tiles (one instruction per
  k-chunk, both heads) - ~96us, the engine's only job.
- DVE: PSUM->SBUF staging (bf16 outputs), causal masking via precomputed
  staircase mask multiply, softmax normalization mul.
- Pool: reciprocal row broadcast (partition_broadcast) + mask building.
- Softmax denominators come from an all-ones 65th column in the V stationary;
  no partition-dim reduce is ever needed.

Dtypes: x/W_S/K/Q/V/exp in bf16 (~0.4% rel err paths), W_NS in fp8-e4m3
(affects only the 64 NS query rows, ~3% there, ~0.6% overall), W_out fp32r,
output bf16 partials summed on host in fp32. Measured end-to-end rel err vs
the fp32 reference: ~1.5e-3 (budget 2e-2).
"""

import os
import sys
import math
import contextlib
from concurrent.futures import ThreadPoolExecutor

for _p in ("/opt/trn_rl_repo", "/root/.axon_site/_ro/trn_rl_repo"):
    if os.path.isdir(_p) and _p not in sys.path:
        sys.path.insert(0, _p)

import numpy as np
import ml_dtypes

import concourse.bass as bass
import concourse.mybir as mybir
import concourse.tile as tile
from concourse import bacc
from concourse.bass_utils import run_bass_kernel_spmd
from concourse.masks import make_identity

F32 = mybir.dt.float32
F32R = mybir.dt.float32r
BF16 = mybir.dt.bfloat16
F8 = mybir.dt.float8e4

N_CORES = 8
D = 1024
H = 16
DH = 64
HPC = H // N_CORES          # heads per core = 2
O3 = 3 * DH * HPC           # 384 qkv output cols per core
LNS = 64
LS = 4096
QS = 2048                   # query_start
LQ = LS - QS + LNS          # 2112 queries
NCH = D // 128              # 8 contraction chunks
ST = 512                    # s-tile width for projections
QT = 512                    # q-tile width for attention
SCALE = DH ** -0.5
NKC = LS // 128             # 32 S key chunks
GT = 32                     # NS-token group size (PSUM partition batch)
WNS_SCALE = 32.0            # fp8 pre-scale for W_NS (dodges e4m3 subnormals)
lqs = LS - QS               # 2048 S-query columns


def build_program(repeat=1):
    nc = bacc.Bacc("TRN2", target_bir_lowering=False, debug=False,
                   num_devices=N_CORES)

    xt_d = nc.dram_tensor("xt", [128, NCH, LS], BF16, kind="ExternalInput")
    xmask_d = nc.dram_tensor("xmask", [128, 4, 2, LNS, GT], F8,
                             kind="ExternalInput")
    ws_d = nc.dram_tensor("ws", [128, NCH, O3], BF16, kind="ExternalInput")
    wns_d = nc.dram_tensor("wns", [LNS, 128, NCH, O3], F8,
                           kind="ExternalInput")
    wout_d = nc.dram_tensor("wout", [128, D], BF16, kind="ExternalInput")
    vones_d = nc.dram_tensor("vones", [65, 64], F32R, kind="ExternalInput")
    o_d = nc.dram_tensor("o", [LQ, D], BF16, kind="ExternalOutput")

    n_kc = NKC + 1           # + NS chunk
    DR = mybir.MatmulPerfMode.DoubleRow

    with tile.TileContext(nc) as tc:
      for _rep in range(repeat):
        with contextlib.ExitStack() as ctx:
            const = ctx.enter_context(tc.tile_pool(name="const", bufs=1))
            store = ctx.enter_context(tc.tile_pool(name="store", bufs=1))

            # --- constants (ws first: the proj matmuls need it + xt0 only;
            # xmask/wout stream in behind the first x tiles) ---
            ws_sb = const.tile([128, NCH, O3], BF16)
            nc.sync.dma_start(out=ws_sb, in_=ws_d.ap())
            xmask_sb = const.tile([128, 4, 2, LNS, GT], F8)
            wout_sb = const.tile([128, D], BF16)
            ident_sb = const.tile([64, 64], F32)
            make_identity(nc, ident_sb[:, :])

            # causal staircase masks, one per diagonal offset d: keep iff
            # -128*d - k_row + q_col >= 0, replicated over the 2-head slot dim
            ones_sb = const.tile([128, 2, 512], BF16)
            nc.vector.memset(ones_sb[:, :, :], 1.0)
            masks_sb = const.tile([128, 4, 2, 512], BF16)
            for d in range(4):
                nc.gpsimd.affine_select(
                    out=masks_sb[:, d], in_=ones_sb[:, :, :],
                    compare_op=mybir.AluOpType.is_ge, fill=0.0,
                    base=-128 * d, channel_multiplier=-1,
                    pattern=[[0, 2], [1, 512]])
            # prewarm the Act exp table so the 1.3us load is off the
            # critical path
            warm_sb = const.tile([1, 4], F32)
            nc.scalar.activation(
                out=warm_sb[0:1, 0:1], in_=ones_sb[0:1, 0, 0:1],
                func=mybir.ActivationFunctionType.Exp, scale=SCALE)
            # f32r all-ones row at partition 64: stationary for the PE
            # reciprocal-row broadcast (rc lives at partition 64, and matmul
            # requires lhsT/rhs partition bases to match). memset can't emit
            # f32r, so DMA it from a tiny DRAM constant.
            ones64_sb = const.tile([65, 64], F32R)
            nc.sync.dma_start(out=ones64_sb, in_=vones_d.ap())

            # --- persistent activation storage ---
            qt_s = store.tile([128, lqs], BF16)     # Q^T S part (h0 0-63, h1 64-127)
            qt_ns = store.tile([128, LNS], BF16)
            kt_s = store.tile([128, LS], BF16)
            kt_ns = store.tile([128, LNS], BF16)
            v_s = [store.tile([128, NKC, 65], BF16, name=f"v_s{h}")
                   for h in range(2)]
            v_ns = [store.tile([64, 65], BF16, name=f"v_ns{h}") for h in range(2)]
            qkvns_sb = store.tile([64, O3], F32)    # natural-layout NS qkv rows

            # ones columns for the denominator trick
            for h in range(2):
                nc.vector.memset(v_s[h][:, :, 64:65], 1.0)
                nc.vector.memset(v_ns[h][:, 64:65], 1.0)

            # ---------------- NS-token projections (emitted interleaved) ----
            wnspool = ctx.enter_context(tc.tile_pool(name="wnspool", bufs=6))
            psNS = ctx.enter_context(tc.tile_pool(name="psNS", bufs=1,
                                                  space="PSUM"))

            def ns_emitter():
                for g in range(LNS // GT):
                    psn = psNS.tile([128, 512], F32, tag="psNS")
                    for j in range(GT):
                        n = GT * g + j
                        wns_t = wnspool.tile([128, NCH, O3], F8, tag="wns")
                        nc.sync.dma_start(out=wns_t, in_=wns_d.ap()[n])
                        for cp in range(NCH // 2):
                            nc.tensor.matmul(
                                psn[0:GT, 0:O3],
                                lhsT=xmask_sb[:, cp, :, n, :],
                                rhs=wns_t[:, 2 * cp:2 * cp + 2, :],
                                start=(j == 0 and cp == 0),
                                stop=(j == GT - 1 and cp == NCH // 2 - 1),
                                perf_mode=DR)
                        yield
                    nc.vector.tensor_scalar_mul(
                        qkvns_sb[GT * g:GT * (g + 1), :], psn[0:GT, 0:O3],
                        1.0 / WNS_SCALE)
                # finalize: Q_NS^T / K_NS^T via PE transpose, V_NS natural
                for part, dest in ((0, qt_ns), (1, kt_ns)):
                    pst = psNS.tile([128, 512], F32, tag="psNS")
                    nc.tensor.transpose(
                        pst[0:128, 0:64],
                        qkvns_sb[0:64, part * 128:(part + 1) * 128],
                        ident_sb[:, :])
                    nc.vector.tensor_copy(out=dest[:, :], in_=pst[0:128, 0:64])
                for h in range(2):
                    nc.vector.tensor_copy(
                        out=v_ns[h][0:64, 0:64],
                        in_=qkvns_sb[0:64, 256 + h * 64:256 + (h + 1) * 64])
                while True:
                    yield

            ns_gen = ns_emitter()
            ns_left = LNS + 1  # token steps + finalize step

            # ---------------- stage A: S-token projections ----------------
            xpool = ctx.enter_context(tc.tile_pool(name="xpool", bufs=2))

            def load_xt(st):
                t = xpool.tile([128, NCH, ST], BF16, tag="xt")
                nc.sync.dma_start(out=t, in_=xt_d.ap()[:, :, st * ST:(st + 1) * ST])
                return t

            xt_next = load_xt(0)
            with tc.tile_pool(name="psA", bufs=2, space="PSUM") as psA:
                for st in range(LS // ST):
                    s0 = st * ST
                    xt_t = xt_next
                    if st + 1 < LS // ST:
                        xt_next = load_xt(st + 1)
                    if st == 3:
                        # deferred constants; needed from the attention phase
                        nc.sync.dma_start(out=xmask_sb, in_=xmask_d.ap())
                        nc.sync.dma_start(out=wout_sb, in_=wout_d.ap())
                    jobs = [(1, kt_s, s0)]
                    if s0 >= QS:
                        jobs.append((0, qt_s, s0 - QS))
                    for mi, dest, dcol in jobs:
                        ps = psA.tile([128, ST], F32, tag="psA")
                        for ci in range(NCH):
                            nc.tensor.matmul(
                                ps[:, :],
                                lhsT=ws_sb[:, ci, mi * 128:(mi + 1) * 128],
                                rhs=xt_t[:, ci, :],
                                start=(ci == 0), stop=(ci == NCH - 1))
                        nc.vector.tensor_copy(out=dest[:, dcol:dcol + ST],
                                              in_=ps[:, :])
                    # V in natural [s, dh] layout: x^T chunk as stationary
                    # NB: start=True marks the whole 2KB PSUM bank pending-zero,
                    # so only the first write in the bank may set it; later
                    # sub-regions accumulate onto pending-zero (reads as 0).
                    psv = psA.tile([128, 4, 128], F32, tag="psV")
                    for sub in range(ST // 128):
                        for ci in range(NCH):
                            nc.tensor.matmul(
                                psv[:, sub, :],
                                lhsT=xt_t[:, ci, sub * 128:(sub + 1) * 128],
                                rhs=ws_sb[:, ci, 256:384],
                                start=(sub == 0 and ci == 0),
                                stop=(sub == ST // 128 - 1 and ci == NCH - 1),
                                skip_group_check=True)
                    for h in range(2):
                        nc.vector.tensor_copy(
                            out=v_s[h][:, st * 4:(st + 1) * 4, 0:64],
                            in_=psv[:, :, h * 64:(h + 1) * 64])

            # ---------------- main attention loop ----------------
            expool = ctx.enter_context(tc.tile_pool(name="expool", bufs=4))
            nrm = ctx.enter_context(tc.tile_pool(name="nrm", bufs=2))
            avpool = ctx.enter_context(tc.tile_pool(name="avpool", bufs=2))
            outpool = ctx.enter_context(tc.tile_pool(name="outpool", bufs=2))
            psS = ctx.enter_context(tc.tile_pool(name="psS", bufs=2, space="PSUM"))
            psAV = ctx.enter_context(tc.tile_pool(name="psAV", bufs=2, space="PSUM"))
            psO = ctx.enter_context(tc.tile_pool(name="psO", bufs=1, space="PSUM"))

            q_tiles = [(q0, min(QT, LQ - q0)) for q0 in range(0, LQ, QT)]
            for qt_i, (q0, qw) in enumerate(q_tiles):
                is_ns_qt = (q0 >= lqs)
                kc_count = min((QS + q0 + qw - 1) // 128 + 1, n_kc)
                if is_ns_qt:
                    while ns_left > 0:
                        next(ns_gen)
                        ns_left -= 1
                ps_av = [psAV.tile([65, 512], F32, tag="psAV", name=f"av{h}")
                         for h in range(2)]
                base_kc = (QS + q0) // 128

                if not is_ns_qt:
                    for kc in range(kc_count):
                        # diagonal chunk at offset d: columns q < 128d are
                        # fully masked - skip them in scores/exp/mask/AV
                        d = kc - base_kc
                        qlo = 128 * d if d > 0 else 0
                        ps_s = psS.tile([128, 2, 512], F32, tag="psS")
                        for h in range(2):
                            hs = slice(h * 64, (h + 1) * 64)
                            nc.tensor.matmul(
                                ps_s[:, h, qlo:qw],
                                lhsT=kt_s[hs, kc * 128:(kc + 1) * 128],
                                rhs=qt_s[hs, q0 + qlo:q0 + qw],
                                start=True, stop=True)
                        ex = expool.tile([128, 2, 512], BF16, tag="ex")
                        nc.scalar.activation(
                            out=ex[:, :, qlo:qw], in_=ps_s[:, :, qlo:qw],
                            func=mybir.ActivationFunctionType.Exp, scale=SCALE)
                        if d >= 0:
                            nc.vector.tensor_tensor(
                                out=ex[:, :, qlo:qw], in0=ex[:, :, qlo:qw],
                                in1=masks_sb[:, d, :, qlo:qw],
                                op=mybir.AluOpType.mult)
                        for h in range(2):
                            nc.tensor.matmul(
                                ps_av[h][0:65, qlo:qw],
                                lhsT=v_s[h][:, kc, :],
                                rhs=ex[:, h, qlo:qw],
                                start=(kc == 0), stop=(kc == kc_count - 1),
                                skip_group_check=True)
                        if ns_left > 1:
                            next(ns_gen)
                            ns_left -= 1
                else:
                    # NS queries: 8 k-chunks share one PSUM pair per exp
                    for cg in range(math.ceil(kc_count / 8)):
                        chunks = list(range(8 * cg, min(8 * cg + 8, kc_count)))
                        ps_s = psS.tile([128, 2, 512], F32, tag="psS")
                        gw = 64 * len(chunks)
                        for i, kc in enumerate(chunks):
                            kw = LNS if kc == NKC else 128
                            for h in range(2):
                                hs = slice(h * 64, (h + 1) * 64)
                                if kc == NKC:
                                    lh = kt_ns[hs, 0:kw]
                                else:
                                    lh = kt_s[hs, kc * 128:(kc + 1) * 128]
                                nc.tensor.matmul(
                                    ps_s[0:kw, h, 64 * i:64 * i + qw],
                                    lhsT=lh, rhs=qt_ns[hs, 0:qw],
                                    start=(i == 0), stop=(i == len(chunks) - 1),
                                    skip_group_check=True)
                        gkw = LNS if chunks == [NKC] else 128
                        ex = expool.tile([128, 2, 512], BF16, tag="ex")
                        nc.scalar.activation(
                            out=ex[0:gkw, :, 0:gw], in_=ps_s[0:gkw, :, 0:gw],
                            func=mybir.ActivationFunctionType.Exp, scale=SCALE)
                        for i, kc in enumerate(chunks):
                            kw = LNS if kc == NKC else 128
                            if kc == NKC:
                                # NS-NS corner: keep iff q >= k row
                                nc.vector.tensor_tensor(
                                    out=ex[0:kw, :, 64 * i:64 * i + qw],
                                    in0=ex[0:kw, :, 64 * i:64 * i + qw],
                                    in1=masks_sb[0:kw, 0, :, 0:qw],
                                    op=mybir.AluOpType.mult)
                            for h in range(2):
                                v_src = v_ns[h][0:kw, 0:65] if kc == NKC \
                                    else v_s[h][0:kw, kc, 0:65]
                                nc.tensor.matmul(
                                    ps_av[h][0:65, 0:qw],
                                    lhsT=v_src,
                                    rhs=ex[0:kw, h, 64 * i:64 * i + qw],
                                    start=(kc == 0), stop=(kc == kc_count - 1))

                # normalize: reciprocal of the ones-column sums, broadcast
                # across the 64 head-dim partitions, multiply in
                av_cat = avpool.tile([128, 512], BF16, tag="av")
                for h in range(2):
                    rc = nrm.tile([65, 512], F32R, tag="rc")
                    with nc.allow_low_precision(reason="f32r recip row feeds "
                                                "full-rate PE broadcast"):
                        nc.vector.reciprocal(out=rc[64:65, 0:qw],
                                             in_=ps_av[h][64:65, 0:qw])
                    pbc = psO.tile([128, 512], F32, tag="po")
                    nc.tensor.matmul(pbc[0:64, 0:qw],
                                     lhsT=ones64_sb[64:65, 0:64],
                                     rhs=rc[64:65, 0:qw],
                                     start=True, stop=True)
                    bc = nrm.tile([64, 512], F32, tag="bc")
                    nc.vector.tensor_copy(out=bc[0:64, 0:qw],
                                          in_=pbc[0:64, 0:qw])
                    nc.vector.tensor_mul(av_cat[h * 64:(h + 1) * 64, 0:qw],
                                         ps_av[h][0:64, 0:qw],
                                         bc[0:64, 0:qw])

                # W_out partial: both heads in one K=128 stationary
                for qs in range(math.ceil(qw / 128)):
                    qsw = min(128, qw - qs * 128)
                    ot = outpool.tile([128, D], BF16, tag="ot")
                    for e in range(2):
                        po = psO.tile([128, 512], F32, tag="po")
                        nc.tensor.matmul(
                            po[0:qsw, :],
                            lhsT=av_cat[:, qs * 128:qs * 128 + qsw],
                            rhs=wout_sb[:, e * 512:(e + 1) * 512],
                            start=True, stop=True)
                        nc.vector.tensor_copy(out=ot[0:qsw, e * 512:(e + 1) * 512],
                                              in_=po[0:qsw, :])
                    nc.sync.dma_start(
                        out=o_d.ap()[q0 + qs * 128:q0 + qs * 128 + qsw, :],
                        in_=ot[0:qsw, :])

    nc.compile()
    return nc


_NC_CACHE = {}


def _get_program():
    if "nc" not in _NC_CACHE:
        _NC_CACHE["nc"] = build_program()
    return _NC_CACHE["nc"]


def prep_shared(x):
    """Shared (core-independent) input tensors."""
    xs = x[0]
    x2 = xs[:LS].T.reshape(NCH, 128, LS).transpose(1, 0, 2)
    xt = np.ascontiguousarray(x2.astype(ml_dtypes.bfloat16))  # (128, 8, 4096)

    # xmask[p, cp, j, n, t] = x_NS[n, (2cp+j)*128+p] if t == n % GT else 0
    xns = xs[LS:]                                      # (64, 1024)
    xm = np.zeros((128, 4, 2, LNS, GT), dtype=ml_dtypes.float8_e4m3)
    xc = xns.T.reshape(4, 2, 128, LNS).transpose(2, 0, 1, 3)  # (128,4,2,64)
    xc8 = xc.astype(ml_dtypes.float8_e4m3)
    for n in range(LNS):
        xm[:, :, :, n, n % GT] = xc8[:, :, :, n]
    return xt, xm


def _prep_core(c, xt, xm, W_S, W_NS, W_out):
    """Host-side shard prep for core c (heads 2c, 2c+1)."""
    h0 = 2 * c * DH
    cols = np.r_[h0:h0 + HPC * DH,
                 D + h0:D + h0 + HPC * DH,
                 2 * D + h0:2 * D + h0 + HPC * DH]
    ws = W_S[:, cols].reshape(NCH, 128, O3).transpose(1, 0, 2)
    ws = np.ascontiguousarray(ws.astype(ml_dtypes.bfloat16))
    wns = W_NS[:, :, cols].reshape(LNS, NCH, 128, O3).transpose(0, 2, 1, 3)
    wns = np.ascontiguousarray(
        (wns * WNS_SCALE).astype(ml_dtypes.float8_e4m3))
    wout = np.ascontiguousarray(
        W_out[h0:h0 + 2 * DH].astype(ml_dtypes.bfloat16))
    return {"xt": xt, "xmask": xm, "ws": ws, "wns": wns, "wout": wout,
            "vones": np.ones((65, 64), dtype=np.float32)}


def prep_in_maps(x, W_S, W_NS, W_out):
    xt, xm = prep_shared(x)
    with ThreadPoolExecutor(max_workers=N_CORES) as ex:
        return list(ex.map(
            lambda c: _prep_core(c, xt, xm, W_S, W_NS, W_out),
            range(N_CORES)))


def kernel(x, W_S, W_NS, W_out, L_S=None, query_start=None, **_unused):
    x = np.asarray(x, dtype=np.float32)
    W_S = np.asarray(W_S, dtype=np.float32)
    W_NS = np.asarray(W_NS, dtype=np.float32)
    W_out = np.asarray(W_out, dtype=np.float32)
    if L_S is not None:
        assert int(L_S) == LS, f"kernel hardcodes L_S={LS}, got {int(L_S)}"
    if query_start is not None:
        assert int(query_start) == QS, \
            f"kernel hardcodes query_start={QS}, got {int(query_start)}"
    assert x.shape == (1, LS + LNS, D)

    nc = _get_program()
    in_maps = prep_in_maps(x, W_S, W_NS, W_out)

    res = None
    for attempt in range(3):
        try:
            res = run_bass_kernel_spmd(nc, in_maps, list(range(N_CORES)))
            break
        except Exception:
            if attempt == 2:
                raise
            # transient device wedges (NRT_EXEC_UNIT_UNRECOVERABLE) have been
            # observed to clear after the terminal resets the session
            import time
            time.sleep(100)
    out = np.zeros((LQ, D), dtype=np.float32)
    for r in res.results:
        out += np.asarray(r["o"], dtype=np.float32)
    return out.reshape(1, LQ, D)


if __name__ == "__main__":
    rng = np.random.default_rng(0)
    ins = {
        "x": rng.standard_normal((1, LS + LNS, D), dtype=np.float32),
        "W_S": rng.standard_normal((D, 3 * D), dtype=np.float32) * 0.02,
        "W_NS": rng.standard_normal((LNS, D, 3 * D), dtype=np.float32) * 0.02,
        "W_out": rng.standard_normal((D, D), dtype=np.float32) * 0.03,
        "L_S": LS, "query_start": QS,
    }
    out = kernel(**ins)
    print("kernel out shape:", out.shape, "finite:", np.isfinite(out).all())
